# revision 14
# baseline (speedup 1.0000x reference)
"""Causal attention kernel for 8 Trainium2 NeuronCores.

Problem: x[4,4096,1024] @ {Wq,Wk,Wv}[1024,64] (+bias) -> causal attention
with softmax scaled by sqrt(seq)=64 -> out[4,4096,64].

Sharding: 8 cores = (batch b) x (half h). Core (b,h) owns query stripes
{512i+256h : +256}; keys/values cover the full batch per core.

Host-side prep: x is stripe-permuted, transposed, cast to bf16, and laid
out as the exact SBUF image per 512-row group, so every x DMA is a plain
2D transfer (128 descriptors of 8KB -> ~0.5us issue, full 16-engine
striping). Weights [Wk|Wv|Wq] likewise one SBUF-image DMA. bk is folded
away (softmax per-query-constant invariance). The cross-half boundary
bias rides in contraction row 64 of kT (kT[64,:]=hbias, qT[64,:]=1);
boundary score matmuls contract over 65 rows, so no separate bias op
touches the exp critical path.

Attention runs in SUPERSTRIPES of 512 queries (stripes 2s,2s+1):
  - full key-tile pairs (attended by both stripes) at N=512
  - edge quads at N=256: diagonal pair (DVE adds triangular mask) +
    boundary pair (65-row contraction)
  - one ACT exp per [128,1024] psum quad, scale 1/64 fused
  - AV with the softmax denominator accumulated via a ones column:
    superstripes 1,2 use fp8e4 DoubleRow (both key tiles per instr,
    0.5 cyc/col); superstripes 0,3 use bf16 (ss0: early queries attend
    few keys so fp8 v-quantization error would not average out; ss3:
    extra PE density keeps the HAM activity monitor at full clock
    through the projection-free tail)
Emission is software-pipelined (each unit's AV deferred past the next
unit's scores+exp) with projection chunks woven between units.
"""

import sys

sys.path.insert(0, "/opt/trn_rl_repo")

from contextlib import ExitStack

import numpy as np
import ml_dtypes

import concourse.bacc as bacc
import concourse.mybir as mybir
import concourse.tile as tile
from concourse.bass import ds, ts
from concourse.bass_utils import run_bass_kernel_spmd
from concourse.masks import make_identity

B, S, D_IN, D_OUT = 4, 4096, 1024, 64
NB = S // 2
N_CORES = 8
NEG = -100.0
SCALE = 1.0 / 64.0

FP32 = mybir.dt.float32
BF16 = mybir.dt.bfloat16
FP8 = mybir.dt.float8e4

N_KT = S // 128
VW = 66
DR = mybir.MatmulPerfMode.DoubleRow
EXP = mybir.ActivationFunctionType.Exp

AV_BF = {0: True, 1: False, 2: False, 3: True}  # AV dtype per superstripe


def build_program():
    nc = bacc.Bacc("TRN2", target_bir_lowering=False, debug=False)

    # all DRAM tensors are exact SBUF images (plain 2D DMAs)
    xt = nc.declare_dram_parameter("xt", [8 * 128, 8 * 512], BF16, isOutput=False)
    wkvq = nc.declare_dram_parameter("wkvq", [128, 8 * 192], BF16, isOutput=False)
    bpk = nc.declare_dram_parameter("bpk", [128, 66], FP32, isOutput=False)
    krow = nc.declare_dram_parameter("krow", [1, S + NB], BF16, isOutput=False)
    out = nc.declare_dram_parameter("out", [NB, D_OUT], FP32, isOutput=True)

    with tile.TileContext(nc) as tc, ExitStack() as ctx:
        const = ctx.enter_context(tc.tile_pool(name="const", bufs=1))
        pers = ctx.enter_context(tc.tile_pool(name="pers", bufs=1))
        expp = ctx.enter_context(tc.tile_pool(name="expp", bufs=5))
        avsb = ctx.enter_context(tc.tile_pool(name="avsb", bufs=2))
        outp = ctx.enter_context(tc.tile_pool(name="outp", bufs=2))
        ps_p = ctx.enter_context(tc.tile_pool(name="ps_p", bufs=1, space="PSUM"))
        ps_s = ctx.enter_context(tc.tile_pool(name="ps_s", bufs=2, space="PSUM"))
        ps_av = ctx.enter_context(tc.tile_pool(name="ps_av", bufs=1, space="PSUM"))
        ps_sm = ctx.enter_context(tc.tile_pool(name="ps_sm", bufs=1, space="PSUM"))

        # --- persistent tiles ---------------------------------------------
        xTs = pers.tile([128, 64 * 512], BF16)
        kT = pers.tile([65, S], BF16)  # row 64 = hbias (via krow DMA)
        qT = pers.tile([65, NB], BF16)  # row 64 = ones
        vsb_bf = pers.tile([128, N_KT * VW], BF16)  # bf16 v_aug (65 used)
        vsb8 = pers.tile([128, N_KT * 128], FP8)  # fp8 v_aug [2,128] blocks

        # --- DMAs ----------------------------------------------------------
        wkvq_sb = const.tile([128, 8 * 192], BF16)
        bp_sb = const.tile([128, 66], FP32)
        # weights first on the sync queue (it spins up fastest and gates the
        # first projection matmul), split so chunk 0 lands ASAP; small
        # biases on the scalar queue
        nc.sync.dma_start(wkvq_sb[:, ds(0, 2 * 192)], wkvq[:, ds(0, 2 * 192)])
        nc.scalar.dma_start(bp_sb[:], bpk[:, :])
        nc.scalar.dma_start(kT[64:65, :], krow[:, ds(0, S)])
        nc.scalar.dma_start(qT[64:65, :], krow[:, ds(S, NB)])

        def dma_rg(rg, half=None):
            if half is None:
                c0, nch = 0, 8
            else:
                c0, nch = 4 * half, 4
            nc.sync.dma_start(
                xTs[:, ds((8 * rg + c0) * 512, nch * 512)],
                xt[ds(128 * rg, 128), ds(512 * c0, 512 * nch)],
            )

        dma_rg(4, 0)
        nc.sync.dma_start(
            wkvq_sb[:, ds(2 * 192, 6 * 192)], wkvq[:, ds(2 * 192, 6 * 192)]
        )
        dma_rg(4, 1)
        for rg in [0, 1, 5, 2, 6, 3, 7]:
            dma_rg(rg)

        # --- constants -----------------------------------------------------
        ident = const.tile([128, 128], FP32)
        make_identity(nc, ident[:])

        mask2 = const.tile([128, 512], FP32)
        nc.gpsimd.memset(mask2[:], 0.0)
        for j in range(2):
            nc.gpsimd.affine_select(
                out=mask2[:, ds(256 * j, 256)],
                in_=mask2[:, ds(256 * j, 256)],
                compare_op=mybir.AluOpType.is_ge,
                fill=NEG / SCALE,
                base=-128 * j,
                pattern=[[1, 256]],
                channel_multiplier=-1,
            )

        nc.gpsimd.memset(vsb8[:], 0.0)
        vsb8_ones = vsb8[:].rearrange("p (t c) -> p t c", c=128)[:, :, 64:65]
        nc.vector.memset(vsb8_ones, 1.0)
        vsbb_ones = vsb_bf[:].rearrange("p (t c) -> p t c", c=VW)[:, :, 64:65]
        nc.vector.memset(vsbb_ones, 1.0)

        bq_ap = bp_sb[0:64, 64:65]
        bv_ap = bp_sb[:, 0:64]

        # --- row groups (projections) -------------------------------------
        rg_pkv = {}
        rg_pq = {}
        rg_vstage = {}

        def rg_chunk(rg, c):
            if c == 0:
                rg_pkv[rg] = ps_p.tile([128, 512], FP32, tag="ps_kv", name="pkv")
                if rg < 4:
                    rg_pq[rg] = ps_p.tile([64, 512], FP32, tag="ps_q", name="pq")
            xs = xTs[:, ds((8 * rg + c) * 512, 512)]
            nc.tensor.matmul(
                rg_pkv[rg][:],
                wkvq_sb[:, ds(192 * c, 128)],
                xs,
                start=(c == 0),
                stop=(c == 7),
            )
            if rg < 4:
                nc.tensor.matmul(
                    rg_pq[rg][:],
                    wkvq_sb[:, ds(192 * c + 128, 64)],
                    xs,
                    start=(c == 0),
                    stop=(c == 7),
                )

        def rg_epi1(rg):
            pkv = rg_pkv[rg]
            nc.vector.tensor_copy(out=kT[0:64, ds(512 * rg, 512)], in_=pkv[0:64, :])
            vstage = avsb.tile([64, 512], FP32, tag="vstage", name="vstage")
            rg_vstage[rg] = vstage
            nc.vector.tensor_copy(out=vstage[:], in_=pkv[64:128, :])
            if rg < 4:
                nc.vector.tensor_scalar_add(
                    qT[0:64, ds(512 * rg, 512)], rg_pq[rg][:], bq_ap
                )

        def rg_epi2(rg):
            vstage = rg_vstage[rg]
            psv = ps_sm.tile([128, 4 * VW], FP32, tag="ps_sm", name="psv")
            for t in range(4):
                nc.tensor.matmul(
                    psv[:, ds(VW * t, 64)],
                    vstage[:, ts(t, 128)],
                    ident[0:64, 0:64],
                    start=(t == 0),
                    stop=(t == 3),
                    is_transpose=True,
                )
            vsrc = psv[:].rearrange("p (t c) -> p t c", c=VW)[:, :, 0:64]
            vdst_b = vsb_bf[:, ds(VW * 4 * rg, 4 * VW)].rearrange(
                "p (t c) -> p t c", c=VW
            )[:, :, 0:64]
            nc.vector.tensor_copy(out=vdst_b, in_=vsrc)
            if rg not in (3, 7):  # fp8 copy only where fp8 AV reads it
                vdst_8 = vsb8[:, ds(128 * 4 * rg, 4 * 128)].rearrange(
                    "p (t c) -> p t c", c=128
                )[:, :, 0:64]
                nc.vector.tensor_copy(out=vdst_8, in_=vsrc)

        def v8pair(kt0):
            return vsb8[:, ds(128 * kt0, 256)].rearrange("p (u c) -> p u c", u=2)

        # --- attention units ----------------------------------------------
        pending = []
        av_state = {}

        def flush_pending():
            for f in pending:
                f()
            pending.clear()

        def av_flags(ss, n=1):
            st = av_state[ss]
            if st[0] is None:
                st[0] = ps_av.tile([128, 512], FP32, tag="ps_av", name="pav")
            flags = []
            for _ in range(n):
                first = st[1] == 0
                st[1] += 1
                flags.append((first, st[1] == st[2]))
            return st[0], flags

        def score_mm(psq_slice, kt, q_lo, q_n, bound, start, stop):
            p_hi = 65 if bound else 64
            nc.tensor.matmul(
                psq_slice,
                kT[0:p_hi, ts(kt, 128)],
                qT[0:p_hi, ds(q_lo, q_n)],
                start=start,
                stop=stop,
            )

        def unit_full(ss, kt0):
            bf = AV_BF[ss]
            psq = ps_s.tile([128, 1024], FP32, tag="ps_s", name="psq")
            for j in range(2):
                score_mm(psq[:, ds(512 * j, 512)], kt0 + j, 512 * ss, 512, False, True, True)
            et = expp.tile(
                [128, 1024], BF16 if bf else FP8, tag="etb" if bf else "et8", name="et"
            )
            nc.scalar.activation(et[:], psq[:], EXP, bias=0.0, scale=SCALE)

            def av():
                if bf:
                    pav, flags = av_flags(ss, 2)
                    for j in range(2):
                        nc.tensor.matmul(
                            pav[0:65, :],
                            vsb_bf[:, ds(VW * (kt0 + j), 65)],
                            et[:, ds(512 * j, 512)],
                            start=flags[j][0],
                            stop=flags[j][1],
                        )
                else:
                    pav, flags = av_flags(ss, 1)
                    nc.tensor.matmul(
                        pav[:],
                        v8pair(kt0),
                        et[:].rearrange("p (u q) -> p u q", u=2),
                        start=flags[0][0],
                        stop=flags[0][1],
                        perf_mode=DR,
                    )

            pending.append(av)

        def unit_edge(i, pairs):
            ss = i // 2
            bf = AV_BF[ss]
            col = 256 * (i % 2)
            psq = ps_s.tile([128, 1024], FP32, tag="ps_s", name="psq")
            for half, (kt0, kind) in enumerate(pairs):
                for j in range(2):
                    score_mm(
                        psq[:, ds(512 * half + 256 * j, 256)],
                        kt0 + j,
                        256 * i,
                        256,
                        kind == "b",
                        j == 0,
                        j == 1,
                    )
            for half, (kt0, kind) in enumerate(pairs):
                if kind == "d":
                    sl = psq[:, ds(512 * half, 512)]
                    nc.vector.tensor_add(sl, sl, mask2[:])
            et = expp.tile(
                [128, 1024], BF16 if bf else FP8, tag="etb" if bf else "et8", name="et"
            )
            nc.scalar.activation(et[:], psq[:], EXP, bias=0.0, scale=SCALE)

            def av():
                if bf:
                    pav, flags = av_flags(ss, 4)
                    n = 0
                    for half, (kt0, kind) in enumerate(pairs):
                        for j in range(2):
                            nc.tensor.matmul(
                                pav[0:65, ds(col, 256)],
                                vsb_bf[:, ds(VW * (kt0 + j), 65)],
                                et[:, ds(512 * half + 256 * j, 256)],
                                start=flags[n][0],
                                stop=flags[n][1],
                            )
                            n += 1
                else:
                    pav, flags = av_flags(ss, 2)
                    for half, (kt0, kind) in enumerate(pairs):
                        nc.tensor.matmul(
                            pav[:, ds(col, 256)],
                            v8pair(kt0),
                            et[:, ds(512 * half, 512)].rearrange(
                                "p (u q) -> p u q", u=2
                            ),
                            start=flags[half][0],
                            stop=flags[half][1],
                            perf_mode=DR,
                        )

            pending.append(av)

        def close_ss(ss, t0=0, nt=4):
            def go():
                pav = av_state[ss][0]
                av = avsb.tile([66, 128 * nt], FP32, tag="av", name="av")
                nc.vector.tensor_copy(out=av[:], in_=pav[0:66, ds(128 * t0, 128 * nt)])
                pso = ps_sm.tile([128, 4 * VW], FP32, tag="ps_sm", name="pso")
                for n in range(nt):
                    t = t0 + n
                    nc.tensor.matmul(
                        pso[:, ds(VW * t, 66)],
                        av[:, ts(n, 128)],
                        ident[0:66, 0:66],
                        start=True,
                        stop=True,
                        is_transpose=True,
                    )
                rec = outp.tile([128, 4], FP32, tag="rec", name="rec")
                ot = outp.tile([128, 4 * 64], FP32, tag="ot", name="ot")
                # one strided reciprocal covers all denominators of this close
                nc.vector.reciprocal(
                    rec[:, ds(t0, nt)].rearrange("p (n o) -> p n o", o=1),
                    pso[:].rearrange("p (t c) -> p t c", c=VW)[:, t0 : t0 + nt, 64:65],
                )
                for n in range(nt):
                    t = t0 + n
                    nc.vector.scalar_tensor_tensor(
                        out=ot[:, ds(64 * t, 64)],
                        in0=pso[:, ds(VW * t, 64)],
                        scalar=rec[:, ds(t, 1)],
                        in1=bv_ap,
                        op0=mybir.AluOpType.mult,
                        op1=mybir.AluOpType.add,
                    )
                nc.sync.dma_start(
                    out[ds(512 * ss + 128 * t0, 128 * nt), :].rearrange(
                        "(u p) o -> p u o", p=128
                    ),
                    ot[:, ds(64 * t0, 64 * nt)].rearrange("p (u o) -> p u o", u=nt),
                )

            pending.append(go)

        # --- emission schedule --------------------------------------------
        for rg in (4, 0):
            for c in range(8):
                rg_chunk(rg, c)
            rg_epi1(rg)
        rg_epi2(4)
        rg_epi2(0)

        def W(rg, *cs):
            return [lambda rg=rg, c=c: rg_chunk(rg, c) for c in cs]

        def WE1(rg):
            return [lambda rg=rg: rg_epi1(rg)]

        def WE2(rg):
            return [lambda rg=rg: rg_epi2(rg)]

        sched = [
            ("E", 0, [(0, "d"), (16, "b")], W(1, 0, 1)),
            ("E", 1, [(0, "n"), (16, "n")], W(1, 2, 3)),
            ("E", 1, [(2, "d"), (18, "b")], W(1, 4, 5, 6, 7) + WE1(1)),
            ("CLOSE", 0),
            ("F", 1, 0, W(5, 0, 1, 2)),
            ("F", 1, 2, W(5, 3, 4, 5)),
            ("F", 1, 16, W(5, 6, 7) + WE1(5)),
            ("F", 1, 18, WE2(1) + W(2, 0)),
            ("E", 2, [(4, "d"), (20, "b")], W(2, 1, 2, 3)),
            ("E", 3, [(4, "n"), (20, "n")], W(2, 4, 5, 6)),
            ("E", 3, [(6, "d"), (22, "b")], W(2, 7) + WE1(2) + WE2(5)),
            ("CLOSE", 1),
            ("F", 2, 0, W(6, 0, 1)),
            ("F", 2, 2, W(6, 2, 3)),
            ("F", 2, 4, W(6, 4, 5)),
            ("F", 2, 6, W(6, 6, 7)),
            ("F", 2, 16, WE1(6) + WE2(2)),
            ("F", 2, 18, WE2(6)),
            ("F", 2, 20, W(3, 0, 1)),
            ("F", 2, 22, W(3, 2, 3)),
            ("E", 4, [(8, "d"), (24, "b")], W(3, 4, 5)),
            ("E", 5, [(8, "n"), (24, "n")], W(3, 6, 7)),
            ("E", 5, [(10, "d"), (26, "b")], WE1(3)),
            ("CLOSE", 2),
            # F units are PE-denser than the exp pace, so they need no weave;
            # rg7's chunks ride the later units to keep PE fed near the tail
            ("F", 3, 0, WE2(3)),
            ("F", 3, 2, []),
            ("F", 3, 4, []),
            ("F", 3, 6, []),
            ("F", 3, 8, W(7, 0)),
            ("F", 3, 10, W(7, 1)),
            ("F", 3, 16, W(7, 2)),
            ("F", 3, 18, W(7, 3)),
            ("F", 3, 20, W(7, 4, 5)),
            ("F", 3, 22, W(7, 6, 7)),
            ("F", 3, 24, WE1(7)),
            ("F", 3, 26, WE2(7)),
            # stripe 7's edges first so its output half closes early; the
            # stripe-6 diagonal/boundary quad is last with a half-close
            ("E", 7, [(12, "n"), (28, "n")], []),
            ("E", 7, [(14, "d"), (30, "b")], []),
            ("CLOSE", 3, 2, 2),
            ("E", 6, [(12, "d"), (28, "b")], []),
            ("CLOSE", 3, 0, 2),
        ]

        for ent in sched:
            if ent[0] == "F":
                ss = ent[1]
                av_state.setdefault(ss, [None, 0, 0])[2] += 2 if AV_BF[ss] else 1
            elif ent[0] == "E":
                ss = ent[1] // 2
                av_state.setdefault(ss, [None, 0, 0])[2] += 4 if AV_BF[ss] else 2

        for ent in sched:
            if ent[0] == "CLOSE":
                close_ss(*ent[1:])
                continue
            if ent[0] == "F":
                _, ss, kt0, weave = ent
                unit_full(ss, kt0)
            else:
                _, i, pairs, weave = ent
                unit_edge(i, pairs)
            new_av = pending.pop()
            flush_pending()
            pending.append(new_av)
            for wv in weave:
                wv()
        flush_pending()

    return nc


_program = None


def _get_program():
    global _program
    if _program is None:
        _program = build_program()
        _program.finalize()
    return _program


def build_in_maps(x, Wq, bq, Wk, bk, Wv, bv):
    x = np.asarray(x, dtype=np.float32)
    Wq = np.asarray(Wq, dtype=np.float32)
    bq = np.asarray(bq, dtype=np.float32)
    Wk = np.asarray(Wk, dtype=np.float32)
    Wv = np.asarray(Wv, dtype=np.float32)
    bv = np.asarray(bv, dtype=np.float32)

    bf = ml_dtypes.bfloat16
    wcat = np.concatenate([Wk, Wv, Wq], axis=1)  # [1024, 192]
    # SBUF image: [128, c*192+o] = wcat[128c+p, o]
    wkvq_np = np.ascontiguousarray(
        wcat.reshape(8, 128, 192).transpose(1, 0, 2).reshape(128, 8 * 192)
    ).astype(bf)

    perm = {}
    for h in range(2):
        idx = []
        for i in range(8):
            idx.append(np.arange(512 * i + 256 * h, 512 * i + 256 * h + 256))
        for i in range(8):
            o = 1 - h
            idx.append(np.arange(512 * i + 256 * o, 512 * i + 256 * o + 256))
        perm[h] = np.concatenate(idx)

    in_maps = []
    for c in range(N_CORES):
        b, h = c // 2, c % 2
        xl = x[b][perm[h]]  # [4096, 1024]
        # SBUF image per row group: [rg*128+p, c*512+r] = xl[512rg+r, 128c+p]
        xt_np = np.ascontiguousarray(
            xl.reshape(8, 512, 8, 128).transpose(0, 3, 2, 1).reshape(8 * 128, 8 * 512)
        ).astype(bf)
        bp = np.zeros((128, 66), np.float32)
        bp[:, 0:64] = bv[None, :]
        bp[0:64, 64] = bq
        kr = np.empty((1, S + NB), np.float32)
        kr[0, :S] = 0.0 if h == 1 else NEG / SCALE
        kr[0, S:] = 1.0
        in_maps.append(
            {
                "xt": xt_np,
                "wkvq": wkvq_np,
                "bpk": bp,
                "krow": kr.astype(bf),
            }
        )
    return in_maps


def kernel(x, Wq, bq, Wk, bk, Wv, bv):
    in_maps = build_in_maps(x, Wq, bq, Wk, bk, Wv, bv)
    nc = _get_program()
    res = run_bass_kernel_spmd(nc, in_maps, list(range(N_CORES)))

    out_full = np.empty((B, S, D_OUT), np.float32)
    for c in range(N_CORES):
        b, h = c // 2, c % 2
        o = res.results[c]["out"]
        for i in range(8):
            out_full[b, 512 * i + 256 * h : 512 * i + 256 * h + 256] = o[
                256 * i : 256 * i + 256
            ]
    return out_full


if __name__ == "__main__":
    rng = np.random.default_rng(0)
    inputs = {
        "x": rng.standard_normal((B, S, D_IN), dtype=np.float32),
        "Wq": rng.standard_normal((D_IN, D_OUT), dtype=np.float32) * 0.02,
        "bq": rng.standard_normal(D_OUT, dtype=np.float32) * 0.02,
        "bk": rng.standard_normal(D_OUT, dtype=np.float32) * 0.02,
        "Wk": rng.standard_normal((D_IN, D_OUT), dtype=np.float32) * 0.02,
        "Wv": rng.standard_normal((D_IN, D_OUT), dtype=np.float32) * 0.02,
        "bv": rng.standard_normal(D_OUT, dtype=np.float32) * 0.02,
    }
    o = kernel(**inputs)
    print("kernel output", o.shape, o.dtype, float(np.abs(o).max()))


# revision 21
# speedup vs baseline: 1.0349x; 1.0349x over previous
"""Causal attention kernel for 8 Trainium2 NeuronCores.

Problem: x[4,4096,1024] @ {Wq,Wk,Wv}[1024,64] (+bias) -> causal attention
with softmax scaled by sqrt(seq)=64 -> out[4,4096,64].

Sharding: 8 cores = (batch b) x (half h). Core (b,h) owns query stripes
{512i+256h : +256}; keys/values cover the full batch per core.

Host-side prep: x is stripe-permuted, transposed, cast to bf16, and laid
out as the exact SBUF image per 512-row group, so every x DMA is a plain
2D transfer (128 descriptors of 8KB -> ~0.5us issue, full 16-engine
striping). Weights [Wk|Wv|Wq] likewise one SBUF-image DMA. bk is folded
away (softmax per-query-constant invariance). The cross-half boundary
bias rides in contraction row 64 of kT (kT[64,:]=hbias, qT[64,:]=1);
boundary score matmuls contract over 65 rows, so no separate bias op
touches the exp critical path.

Attention runs in SUPERSTRIPES of 512 queries (stripes 2s,2s+1):
  - full key-tile pairs (attended by both stripes) at N=512
  - edge quads at N=256: diagonal pair (DVE adds triangular mask) +
    boundary pair (65-row contraction)
  - one ACT exp per [128,1024] psum quad, scale 1/64 fused
  - AV with the softmax denominator accumulated via a ones column:
    superstripes 1,2 use fp8e4 DoubleRow (both key tiles per instr,
    0.5 cyc/col); superstripes 0,3 use bf16 (ss0: early queries attend
    few keys so fp8 v-quantization error would not average out; ss3:
    extra PE density keeps the HAM activity monitor at full clock
    through the projection-free tail)
Emission is software-pipelined (each unit's AV deferred past the next
unit's scores+exp) with projection chunks woven between units.
"""

import sys

sys.path.insert(0, "/opt/trn_rl_repo")

from contextlib import ExitStack

import numpy as np
import ml_dtypes

import concourse.bacc as bacc
import concourse.mybir as mybir
import concourse.tile as tile
from concourse.bass import ds, ts
from concourse.bass_utils import run_bass_kernel_spmd
from concourse.masks import make_identity

B, S, D_IN, D_OUT = 4, 4096, 1024, 64
NB = S // 2
N_CORES = 8
NEG = -100.0
SCALE = 1.0 / 64.0

FP32 = mybir.dt.float32
BF16 = mybir.dt.bfloat16
FP8 = mybir.dt.float8e4

N_KT = S // 128
VW = 66
DR = mybir.MatmulPerfMode.DoubleRow
EXP = mybir.ActivationFunctionType.Exp

AV_BF = {0: True, 1: False, 2: False, 3: True}  # AV dtype per superstripe


def build_program():
    nc = bacc.Bacc("TRN2", target_bir_lowering=False, debug=False)

    # all DRAM tensors are exact SBUF images (plain 2D DMAs)
    xt = nc.declare_dram_parameter("xt", [8 * 128, 8 * 512], BF16, isOutput=False)
    wkvq = nc.declare_dram_parameter("wkvq", [128, 8 * 192], BF16, isOutput=False)
    bpk = nc.declare_dram_parameter("bpk", [128, 66], FP32, isOutput=False)
    krow = nc.declare_dram_parameter("krow", [1, S + NB], BF16, isOutput=False)
    out = nc.declare_dram_parameter("out", [NB, D_OUT], FP32, isOutput=True)

    with tile.TileContext(nc) as tc, ExitStack() as ctx:
        const = ctx.enter_context(tc.tile_pool(name="const", bufs=1))
        pers = ctx.enter_context(tc.tile_pool(name="pers", bufs=1))
        expp = ctx.enter_context(tc.tile_pool(name="expp", bufs=5))
        avsb = ctx.enter_context(tc.tile_pool(name="avsb", bufs=2))
        outp = ctx.enter_context(tc.tile_pool(name="outp", bufs=2))
        # bank budget (8): ps_s 3x[128,1024] = 6, ps_p 1 (kv and q phases
        # rotate through the same slot), ps_av 1. The v-transpose and
        # epilogue-transpose scratch live in ps_s slots.
        ps_p = ctx.enter_context(tc.tile_pool(name="ps_p", bufs=1, space="PSUM"))
        ps_s = ctx.enter_context(tc.tile_pool(name="ps_s", bufs=3, space="PSUM"))
        ps_av = ctx.enter_context(tc.tile_pool(name="ps_av", bufs=1, space="PSUM"))

        # --- persistent tiles ---------------------------------------------
        xTs = pers.tile([128, 64 * 512], BF16)
        kT = pers.tile([65, S], BF16)  # row 64 = hbias (via krow DMA)
        qT = pers.tile([65, NB], BF16)  # row 64 = ones
        vsb_bf = pers.tile([128, N_KT * VW], BF16)  # bf16 v_aug (65 used)
        vsb8 = pers.tile([128, N_KT * 128], FP8)  # fp8 v_aug [2,128] blocks

        # --- DMAs ----------------------------------------------------------
        wkvq_sb = const.tile([128, 8 * 192], BF16)
        bp_sb = const.tile([128, 66], FP32)
        # weights first on the sync queue (it spins up fastest and gates the
        # first projection matmul); small biases on the scalar queue
        nc.sync.dma_start(wkvq_sb[:], wkvq[:, :])
        nc.scalar.dma_start(bp_sb[:], bpk[:, :])
        nc.scalar.dma_start(kT[64:65, :], krow[:, ds(0, S)])
        nc.scalar.dma_start(qT[64:65, :], krow[:, ds(S, NB)])

        def dma_rg(rg):
            nc.sync.dma_start(
                xTs[:, ds(8 * 512 * rg, 8 * 512)], xt[ds(128 * rg, 128), :]
            )

        for rg in [4, 0, 1, 5, 2, 6, 3, 7]:
            dma_rg(rg)

        # --- constants -----------------------------------------------------
        ident = const.tile([128, 128], FP32)
        make_identity(nc, ident[:])

        mask2 = const.tile([128, 512], FP32)
        nc.gpsimd.memset(mask2[:], 0.0)
        for j in range(2):
            nc.gpsimd.affine_select(
                out=mask2[:, ds(256 * j, 256)],
                in_=mask2[:, ds(256 * j, 256)],
                compare_op=mybir.AluOpType.is_ge,
                fill=NEG / SCALE,
                base=-128 * j,
                pattern=[[1, 256]],
                channel_multiplier=-1,
            )

        nc.gpsimd.memset(vsb8[:], 0.0)
        vsb8_ones = vsb8[:].rearrange("p (t c) -> p t c", c=128)[:, :, 64:65]
        nc.vector.memset(vsb8_ones, 1.0)
        vsbb_ones = vsb_bf[:].rearrange("p (t c) -> p t c", c=VW)[:, :, 64:65]
        nc.vector.memset(vsbb_ones, 1.0)

        bq_ap = bp_sb[0:64, 64:65]
        bv_ap = bp_sb[:, 0:64]

        # --- row groups (projections) -------------------------------------
        rg_pkv = {}
        rg_pq = {}
        rg_vstage = {}

        def rg_chunk(rg, c):
            if c == 0:
                rg_pkv[rg] = ps_p.tile([128, 512], FP32, tag="ps_p", name="pkv")
            xs = xTs[:, ds((8 * rg + c) * 512, 512)]
            nc.tensor.matmul(
                rg_pkv[rg][:],
                wkvq_sb[:, ds(192 * c, 128)],
                xs,
                start=(c == 0),
                stop=(c == 7),
            )

        def rg_qchunk(rg, c):
            if c == 0:
                rg_pq[rg] = ps_p.tile([64, 512], FP32, tag="ps_p", name="pq")
            xs = xTs[:, ds((8 * rg + c) * 512, 512)]
            nc.tensor.matmul(
                rg_pq[rg][:],
                wkvq_sb[:, ds(192 * c + 128, 64)],
                xs,
                start=(c == 0),
                stop=(c == 7),
            )

        def rg_epi1a(rg):
            pkv = rg_pkv[rg]
            nc.vector.tensor_copy(out=kT[0:64, ds(512 * rg, 512)], in_=pkv[0:64, :])
            vstage = avsb.tile([64, 512], FP32, tag="vstage", name="vstage")
            rg_vstage[rg] = vstage
            nc.vector.tensor_copy(out=vstage[:], in_=pkv[64:128, :])

        def rg_epi1b(rg):
            nc.vector.tensor_scalar_add(
                qT[0:64, ds(512 * rg, 512)], rg_pq[rg][:], bq_ap
            )

        def rg_epi2(rg):
            vstage = rg_vstage[rg]
            psv = ps_s.tile([128, 1024], FP32, tag="ps_s", name="psv")
            for t in range(4):
                nc.tensor.matmul(
                    psv[:, ds(VW * t, 64)],
                    vstage[:, ts(t, 128)],
                    ident[0:64, 0:64],
                    start=(t == 0),
                    stop=(t == 3),
                    is_transpose=True,
                )
            vsrc = psv[:, ds(0, 4 * VW)].rearrange("p (t c) -> p t c", c=VW)[
                :, :, 0:64
            ]
            vdst_b = vsb_bf[:, ds(VW * 4 * rg, 4 * VW)].rearrange(
                "p (t c) -> p t c", c=VW
            )[:, :, 0:64]
            nc.vector.tensor_copy(out=vdst_b, in_=vsrc)
            if rg not in (3, 7):  # fp8 copy only where fp8 AV reads it
                vdst_8 = vsb8[:, ds(128 * 4 * rg, 4 * 128)].rearrange(
                    "p (t c) -> p t c", c=128
                )[:, :, 0:64]
                nc.vector.tensor_copy(out=vdst_8, in_=vsrc)

        def v8pair(kt0):
            return vsb8[:, ds(128 * kt0, 256)].rearrange("p (u c) -> p u c", u=2)

        # --- attention units ----------------------------------------------
        pending = []
        av_state = {}

        def flush_pending():
            for f in pending:
                f()
            pending.clear()

        def av_flags(ss, n=1):
            st = av_state[ss]
            if st[0] is None:
                st[0] = ps_av.tile([128, 512], FP32, tag="ps_av", name="pav")
            flags = []
            for _ in range(n):
                first = st[1] == 0
                st[1] += 1
                flags.append((first, st[1] == st[2]))
            return st[0], flags

        def score_mm(psq_slice, kt, q_lo, q_n, bound, start, stop):
            p_hi = 65 if bound else 64
            nc.tensor.matmul(
                psq_slice,
                kT[0:p_hi, ts(kt, 128)],
                qT[0:p_hi, ds(q_lo, q_n)],
                start=start,
                stop=stop,
            )

        def unit_full(ss, kt0):
            bf = AV_BF[ss]
            psq = ps_s.tile([128, 1024], FP32, tag="ps_s", name="psq")
            for j in range(2):
                score_mm(psq[:, ds(512 * j, 512)], kt0 + j, 512 * ss, 512, False, True, True)
            et = expp.tile(
                [128, 1024], BF16 if bf else FP8, tag="etb" if bf else "et8", name="et"
            )
            nc.scalar.activation(et[:], psq[:], EXP, bias=0.0, scale=SCALE)

            def av():
                if bf:
                    pav, flags = av_flags(ss, 2)
                    for j in range(2):
                        nc.tensor.matmul(
                            pav[0:65, :],
                            vsb_bf[:, ds(VW * (kt0 + j), 65)],
                            et[:, ds(512 * j, 512)],
                            start=flags[j][0],
                            stop=flags[j][1],
                        )
                else:
                    pav, flags = av_flags(ss, 1)
                    nc.tensor.matmul(
                        pav[:],
                        v8pair(kt0),
                        et[:].rearrange("p (u q) -> p u q", u=2),
                        start=flags[0][0],
                        stop=flags[0][1],
                        perf_mode=DR,
                    )

            pending.append(av)

        def unit_edge(i, pairs):
            ss = i // 2
            bf = AV_BF[ss]
            col = 256 * (i % 2)
            psq = ps_s.tile([128, 1024], FP32, tag="ps_s", name="psq")
            for half, (kt0, kind) in enumerate(pairs):
                for j in range(2):
                    score_mm(
                        psq[:, ds(512 * half + 256 * j, 256)],
                        kt0 + j,
                        256 * i,
                        256,
                        kind == "b",
                        j == 0,
                        j == 1,
                    )
            for half, (kt0, kind) in enumerate(pairs):
                if kind == "d":
                    sl = psq[:, ds(512 * half, 512)]
                    nc.vector.tensor_add(sl, sl, mask2[:])
            et = expp.tile(
                [128, 1024], BF16 if bf else FP8, tag="etb" if bf else "et8", name="et"
            )
            nc.scalar.activation(et[:], psq[:], EXP, bias=0.0, scale=SCALE)

            def av():
                if bf:
                    pav, flags = av_flags(ss, 4)
                    n = 0
                    for half, (kt0, kind) in enumerate(pairs):
                        for j in range(2):
                            nc.tensor.matmul(
                                pav[0:65, ds(col, 256)],
                                vsb_bf[:, ds(VW * (kt0 + j), 65)],
                                et[:, ds(512 * half + 256 * j, 256)],
                                start=flags[n][0],
                                stop=flags[n][1],
                            )
                            n += 1
                else:
                    pav, flags = av_flags(ss, 2)
                    for half, (kt0, kind) in enumerate(pairs):
                        nc.tensor.matmul(
                            pav[:, ds(col, 256)],
                            v8pair(kt0),
                            et[:, ds(512 * half, 512)].rearrange(
                                "p (u q) -> p u q", u=2
                            ),
                            start=flags[half][0],
                            stop=flags[half][1],
                            perf_mode=DR,
                        )

            pending.append(av)

        def close_ss(ss, t0=0, nt=4):
            def go():
                pav = av_state[ss][0]
                av = avsb.tile([66, 128 * nt], FP32, tag="av", name="av")
                nc.vector.tensor_copy(out=av[:], in_=pav[0:66, ds(128 * t0, 128 * nt)])
                pso = ps_s.tile([128, 1024], FP32, tag="ps_s", name="pso")
                for n in range(nt):
                    t = t0 + n
                    nc.tensor.matmul(
                        pso[:, ds(VW * t, 66)],
                        av[:, ts(n, 128)],
                        ident[0:66, 0:66],
                        start=True,
                        stop=True,
                        is_transpose=True,
                    )
                rec = outp.tile([128, 4], FP32, tag="rec", name="rec")
                ot = outp.tile([128, 4 * 64], FP32, tag="ot", name="ot")
                # one strided reciprocal covers all denominators of this close
                nc.vector.reciprocal(
                    rec[:, ds(t0, nt)].rearrange("p (n o) -> p n o", o=1),
                    pso[:, ds(0, 4 * VW)].rearrange("p (t c) -> p t c", c=VW)[
                        :, t0 : t0 + nt, 64:65
                    ],
                )
                for n in range(nt):
                    t = t0 + n
                    nc.vector.scalar_tensor_tensor(
                        out=ot[:, ds(64 * t, 64)],
                        in0=pso[:, ds(VW * t, 64)],
                        scalar=rec[:, ds(t, 1)],
                        in1=bv_ap,
                        op0=mybir.AluOpType.mult,
                        op1=mybir.AluOpType.add,
                    )
                nc.sync.dma_start(
                    out[ds(512 * ss + 128 * t0, 128 * nt), :].rearrange(
                        "(u p) o -> p u o", p=128
                    ),
                    ot[:, ds(64 * t0, 64 * nt)].rearrange("p (u o) -> p u o", u=nt),
                )

            pending.append(go)

        # --- emission schedule --------------------------------------------
        # prologue: rg4 (kv only) and rg0's full chain (kv then q through
        # the same psum slot); everything else woven between attention units
        for c in range(8):
            rg_chunk(4, c)
        rg_epi1a(4)
        for c in range(8):
            rg_chunk(0, c)
        rg_epi1a(0)
        rg_epi2(4)
        for c in range(8):
            rg_qchunk(0, c)
        rg_epi1b(0)
        rg_epi2(0)

        def W(rg, *cs):
            return [lambda rg=rg, c=c: rg_chunk(rg, c) for c in cs]

        def WQ(rg, *cs):
            return [lambda rg=rg, c=c: rg_qchunk(rg, c) for c in cs]

        def WA(rg):
            return [lambda rg=rg: rg_epi1a(rg)]

        def WB(rg):
            return [lambda rg=rg: rg_epi1b(rg)]

        def WE2(rg):
            return [lambda rg=rg: rg_epi2(rg)]

        sched = [
            ("E", 0, [(0, "d"), (16, "b")], W(1, 0, 1, 2, 3, 4, 5)),
            ("E", 1, [(0, "n"), (16, "n")], W(1, 6, 7) + WA(1)),
            ("E", 1, [(2, "d"), (18, "b")], WQ(1, 0, 1, 2, 3, 4, 5, 6, 7) + WB(1)),
            ("CLOSE", 0),
            ("F", 1, 0, WE2(1) + W(5, 0, 1, 2)),
            ("F", 1, 2, W(5, 3, 4, 5)),
            ("F", 1, 16, W(5, 6, 7) + WA(5)),
            ("F", 1, 18, W(2, 0, 1, 2, 3)),
            ("E", 2, [(4, "d"), (20, "b")], WE2(5) + W(2, 4, 5)),
            ("E", 3, [(4, "n"), (20, "n")], W(2, 6, 7) + WA(2)),
            ("E", 3, [(6, "d"), (22, "b")], WQ(2, 0, 1, 2, 3, 4, 5, 6, 7) + WB(2)),
            ("CLOSE", 1),
            ("F", 2, 0, WE2(2) + W(6, 0, 1, 2)),
            ("F", 2, 2, W(6, 3, 4, 5)),
            ("F", 2, 4, W(6, 6, 7) + WA(6)),
            ("F", 2, 6, W(3, 0, 1, 2, 3)),
            ("F", 2, 16, WE2(6) + W(3, 4, 5)),
            ("F", 2, 18, W(3, 6, 7) + WA(3)),
            ("F", 2, 20, WQ(3, 0, 1, 2, 3)),
            ("F", 2, 22, WQ(3, 4, 5, 6, 7) + WB(3)),
            ("E", 4, [(8, "d"), (24, "b")], WE2(3)),
            ("E", 5, [(8, "n"), (24, "n")], []),
            ("E", 5, [(10, "d"), (26, "b")], []),
            ("CLOSE", 2),
            # F units are PE-denser than the exp pace, so they need no weave;
            # rg7's chunks ride the later units to keep PE fed near the tail
            ("F", 3, 0, []),
            ("F", 3, 2, []),
            ("F", 3, 4, []),
            ("F", 3, 6, []),
            ("F", 3, 8, W(7, 0, 1)),
            ("F", 3, 10, W(7, 2, 3)),
            ("F", 3, 16, W(7, 4, 5)),
            ("F", 3, 18, W(7, 6, 7)),
            ("F", 3, 20, WA(7)),
            ("F", 3, 22, []),
            ("F", 3, 24, WE2(7)),
            ("F", 3, 26, []),
            # stripe 7's edges first so its output half closes early; the
            # stripe-6 diagonal/boundary quad is last with a half-close
            ("E", 7, [(12, "n"), (28, "n")], []),
            ("E", 7, [(14, "d"), (30, "b")], []),
            ("CLOSE", 3, 2, 2),
            ("E", 6, [(12, "d"), (28, "b")], []),
            ("CLOSE", 3, 0, 2),
        ]

        for ent in sched:
            if ent[0] == "F":
                ss = ent[1]
                av_state.setdefault(ss, [None, 0, 0])[2] += 2 if AV_BF[ss] else 1
            elif ent[0] == "E":
                ss = ent[1] // 2
                av_state.setdefault(ss, [None, 0, 0])[2] += 4 if AV_BF[ss] else 2

        for ent in sched:
            if ent[0] == "CLOSE":
                close_ss(*ent[1:])
                continue
            if ent[0] == "F":
                _, ss, kt0, weave = ent
                unit_full(ss, kt0)
            else:
                _, i, pairs, weave = ent
                unit_edge(i, pairs)
            new_av = pending.pop()
            flush_pending()
            pending.append(new_av)
            for wv in weave:
                wv()
        flush_pending()

    return nc


_program = None


def _get_program():
    global _program
    if _program is None:
        _program = build_program()
        _program.finalize()
    return _program


def build_in_maps(x, Wq, bq, Wk, bk, Wv, bv):
    x = np.asarray(x, dtype=np.float32)
    Wq = np.asarray(Wq, dtype=np.float32)
    bq = np.asarray(bq, dtype=np.float32)
    Wk = np.asarray(Wk, dtype=np.float32)
    Wv = np.asarray(Wv, dtype=np.float32)
    bv = np.asarray(bv, dtype=np.float32)

    bf = ml_dtypes.bfloat16
    wcat = np.concatenate([Wk, Wv, Wq], axis=1)  # [1024, 192]
    # SBUF image: [128, c*192+o] = wcat[128c+p, o]
    wkvq_np = np.ascontiguousarray(
        wcat.reshape(8, 128, 192).transpose(1, 0, 2).reshape(128, 8 * 192)
    ).astype(bf)

    perm = {}
    for h in range(2):
        idx = []
        for i in range(8):
            idx.append(np.arange(512 * i + 256 * h, 512 * i + 256 * h + 256))
        for i in range(8):
            o = 1 - h
            idx.append(np.arange(512 * i + 256 * o, 512 * i + 256 * o + 256))
        perm[h] = np.concatenate(idx)

    in_maps = []
    for c in range(N_CORES):
        b, h = c // 2, c % 2
        xl = x[b][perm[h]]  # [4096, 1024]
        # SBUF image per row group: [rg*128+p, c*512+r] = xl[512rg+r, 128c+p]
        xt_np = np.ascontiguousarray(
            xl.reshape(8, 512, 8, 128).transpose(0, 3, 2, 1).reshape(8 * 128, 8 * 512)
        ).astype(bf)
        bp = np.zeros((128, 66), np.float32)
        bp[:, 0:64] = bv[None, :]
        bp[0:64, 64] = bq
        kr = np.empty((1, S + NB), np.float32)
        kr[0, :S] = 0.0 if h == 1 else NEG / SCALE
        kr[0, S:] = 1.0
        in_maps.append(
            {
                "xt": xt_np,
                "wkvq": wkvq_np,
                "bpk": bp,
                "krow": kr.astype(bf),
            }
        )
    return in_maps


def kernel(x, Wq, bq, Wk, bk, Wv, bv):
    in_maps = build_in_maps(x, Wq, bq, Wk, bk, Wv, bv)
    nc = _get_program()
    res = run_bass_kernel_spmd(nc, in_maps, list(range(N_CORES)))

    out_full = np.empty((B, S, D_OUT), np.float32)
    for c in range(N_CORES):
        b, h = c // 2, c % 2
        o = res.results[c]["out"]
        for i in range(8):
            out_full[b, 512 * i + 256 * h : 512 * i + 256 * h + 256] = o[
                256 * i : 256 * i + 256
            ]
    return out_full


if __name__ == "__main__":
    rng = np.random.default_rng(0)
    inputs = {
        "x": rng.standard_normal((B, S, D_IN), dtype=np.float32),
        "Wq": rng.standard_normal((D_IN, D_OUT), dtype=np.float32) * 0.02,
        "bq": rng.standard_normal(D_OUT, dtype=np.float32) * 0.02,
        "bk": rng.standard_normal(D_OUT, dtype=np.float32) * 0.02,
        "Wk": rng.standard_normal((D_IN, D_OUT), dtype=np.float32) * 0.02,
        "Wv": rng.standard_normal((D_IN, D_OUT), dtype=np.float32) * 0.02,
        "bv": rng.standard_normal(D_OUT, dtype=np.float32) * 0.02,
    }
    o = kernel(**inputs)
    print("kernel output", o.shape, o.dtype, float(np.abs(o).max()))
